# revision 1
# baseline (speedup 1.0000x reference)
"""Trainium2 Bass kernel for NNConv-style GNN message passing.

Math (edge_attr == ones):
  h   = relu(x @ lin0_w + lin0_b)                      [N, 32]
  W   = (relu(nn_w1[0] + nn_b1) @ nn_w2 + nn_b2).reshape(32, 32)  (constant!)
  g0  = segment_sum(h[src], dst, N)                    [N, 32]
  out = g0 @ W + h @ conv_root + conv_bias             [N, 32]
  edge_emb = relu((out[src] * out[dst]) @ lin1_w + lin1_b)
  score    = edge_emb @ lin2_w + lin2_b                [E]

Mapping to 8 NeuronCores (SPMD, one shared program):
  * edges sorted by dst; core c owns dst in [c*6250, (c+1)*6250)
  * dst side (aggregation in phase 1, out[dst] expansion in phase 2) is
    done ON-CHIP via one-hot matmuls into PSUM (zero DMA descriptors)
  * src side is the only true random crossing: dma_gather of 256B pair
    rows (h or out for nodes 2p, 2p+1) + parity select on DVE
  * node MLPs on PE with PE-transposes; h/out tables AllGather'd
"""
import numpy as np

N_NODES = 50000
N_EDGES = 400000
IN_FEAT = 64
H_DIM = 32
N_CORES = 8
NPC = N_NODES // N_CORES        # 6250 nodes per core
BLOCKS = (NPC + 127) // 128     # 49 dst blocks per core
NPAIR = N_NODES // 2
P = 128
OP_T = 7                        # tiles per gather op (896 idx, ring-safe)
LAST_BLK_N = NPC - (BLOCKS - 1) * 128   # 106


def _prep(x, edge_index):
    """Host-side sharding/sorting. Returns per-core arrays + structure."""
    src = np.asarray(edge_index[0]).astype(np.int64)
    dst = np.asarray(edge_index[1]).astype(np.int64)
    E = src.size
    order = np.argsort(dst, kind="stable")
    s_s, d_s = src[order], dst[order]
    core = d_s // NPC
    d_loc = d_s - core * NPC
    blk = d_loc // 128
    key = core * BLOCKS + blk
    counts = np.bincount(key, minlength=N_CORES * BLOCKS)
    T_B = max(1, int(np.ceil(counts.max() / 128)))
    T_total = BLOCKS * T_B
    n_ops = -(-T_total // OP_T)
    T_pad = n_ops * OP_T
    SLOTS = T_pad * 128

    starts = np.zeros(N_CORES * BLOCKS + 1, np.int64)
    np.cumsum(counts, out=starts[1:])
    pos_in_blk = np.arange(E) - starts[key]
    slot = blk * (T_B * 128) + pos_in_blk     # slot within the core

    sp = np.zeros((N_CORES, SLOTS), np.int64)         # src pair idx (sentinel 0)
    bs = np.zeros((N_CORES, SLOTS), np.uint8)         # src parity
    dr = np.full((N_CORES, SLOTS), -1.0, np.float32)  # dst rel in block
    inv = np.full((N_CORES, SLOTS), -1, np.int64)     # original edge id
    sp[core, slot] = s_s // 2
    bs[core, slot] = (s_s & 1).astype(np.uint8)
    dr[core, slot] = (d_loc - blk * 128).astype(np.float32)
    inv[core, slot] = order

    # gather idx wrap layout: per op of 896, idx k -> [k%16, k//16],
    # replicated across the 8 Q7 cores (128 partitions)
    gsrc = np.empty((N_CORES, 128, SLOTS // 16), np.int16)
    for c in range(N_CORES):
        a = sp[c].reshape(n_ops, OP_T * 128 // 16, 16).transpose(0, 2, 1)
        w16 = a.transpose(1, 0, 2).reshape(16, SLOTS // 16)
        gsrc[c] = np.tile(w16.astype(np.int16), (8, 1))

    # [128, T_pad] layouts (p, t)
    bs_pt = bs.reshape(N_CORES, T_pad, 128).transpose(0, 2, 1).copy()
    dr_pt = dr.reshape(N_CORES, T_pad, 128).transpose(0, 2, 1).copy()
    # per-block transposed dst_rel rows for phase 2: [BLOCKS, T_B*128]
    drT = dr[:, :T_total * 128].reshape(N_CORES, BLOCKS, T_B * 128).copy()

    xs = np.asarray(x, np.float32)
    x_sh = np.zeros((N_CORES, BLOCKS * 128, IN_FEAT), np.float32)
    x_sh[:, :NPC] = xs.reshape(N_CORES, NPC, IN_FEAT)

    return dict(T_B=T_B, T_total=T_total, n_ops=n_ops, T_pad=T_pad,
                SLOTS=SLOTS, gsrc=gsrc, bs_pt=bs_pt, dr_pt=dr_pt,
                drT=drT, x_sh=x_sh, inv=inv)


def _weights(ins, T_B):
    f32 = np.float32
    g = {k: np.asarray(v) for k, v in ins.items()}
    v = np.maximum(g["nn_w1"][0] + g["nn_b1"], 0.0)
    W = (v @ g["nn_w2"] + g["nn_b2"]).reshape(H_DIM, H_DIM)
    Wcat = np.concatenate([W, g["conv_root"]], 0).astype(f32)     # [64, 32]
    w0 = g["lin0_w"].astype(f32)                                  # [64, 32]
    b0 = np.tile(g["lin0_b"][None, :], (P, 1)).astype(f32)        # [128, 32]
    cb = np.tile(g["conv_bias"][None, :], (P, 1)).astype(f32)     # [128, 32]
    w1 = np.tile(g["lin1_w"].astype(f32), (4, 1))                 # [128, 8]
    b1 = np.tile(np.tile(g["lin1_b"], T_B)[None, :], (P, 1)).astype(f32)
    w2 = np.tile(np.tile(g["lin2_w"][:, 0], T_B)[None, :], (P, 1)).astype(f32)
    b2 = np.full((P, 1), float(g["lin2_b"].reshape(-1)[0]), f32)
    iota_f = np.tile(np.arange(P, dtype=f32)[None, :], (P, 1))    # [128,128]
    iota_p = np.ascontiguousarray(iota_f.T)
    ones1 = np.ones((1, P), f32)
    return dict(Wcat=Wcat, w0=w0, b0=b0, cb=cb, w1=w1, b1=b1, w2=w2, b2=b2,
                iota_f=iota_f, iota_p=iota_p, ones1=ones1)


def _build(T_B, n_ops, k_rep=1, phases=3, p2step=99):
    import concourse.bacc as bacc
    import concourse.mybir as mybir
    import concourse.tile as tile
    from concourse.library_config import mlp
    from concourse.masks import make_identity

    f32 = mybir.dt.float32
    T_pad = n_ops * OP_T
    T_total = BLOCKS * T_B
    SLOTS = T_pad * 128

    nc = bacc.Bacc("TRN2", target_bir_lowering=False, debug=False,
                   num_devices=N_CORES)
    dt = nc.dram_tensor
    x_d = dt("x_sh", [BLOCKS * 128, IN_FEAT], f32, kind="ExternalInput")
    gsrc_d = dt("gsrc", [128, SLOTS // 16], mybir.dt.int16,
                kind="ExternalInput")
    bs_d = dt("bs_pt", [128, T_pad], mybir.dt.uint8,
               kind="ExternalInput")
    dr_d = dt("dr_pt", [128, T_pad], f32, kind="ExternalInput")
    drT_d = dt("drT", [BLOCKS, T_B * 128], f32, kind="ExternalInput")
    wc_d = dt("Wcat", [64, 32], f32, kind="ExternalInput")
    w0_d = dt("w0", [64, 32], f32, kind="ExternalInput")
    b0_d = dt("b0", [P, 32], f32, kind="ExternalInput")
    cb_d = dt("cb", [P, 32], f32, kind="ExternalInput")
    w1_d = dt("w1", [128, 8], f32, kind="ExternalInput")
    b1_d = dt("b1", [P, T_B * 8], f32, kind="ExternalInput")
    w2_d = dt("w2", [P, T_B * 8], f32, kind="ExternalInput")
    io_f_d = dt("iota_f", [P, P], f32, kind="ExternalInput")
    io_p_d = dt("iota_p", [P, P], f32, kind="ExternalInput")
    on1_d = dt("ones1", [1, P], f32, kind="ExternalInput")
    b2_d = dt("b2", [P, 1], f32, kind="ExternalInput")

    # flat pair tables (contiguous h rows 2i,2i+1 form one 256B pair row)
    h_shard = dt("h_shard", [NPC * H_DIM], f32)
    o_shard = dt("o_shard", [NPC * H_DIM], f32)
    h_full = dt("h_full", [NPAIR, 64], f32, addr_space="Shared")
    o_full = dt("o_full", [NPAIR, 64], f32, addr_space="Shared")
    sc_d = dt("scores", [P, T_pad], f32, kind="ExternalOutput")

    groups = [list(range(N_CORES))]
    bypass = mybir.AluOpType.bypass
    add = mybir.AluOpType.add
    mult = mybir.AluOpType.mult
    iseq = mybir.AluOpType.is_equal
    Relu = mybir.ActivationFunctionType.Relu
    X = mybir.AxisListType.X

    with tile.TileContext(nc) as tc:
        with tc.tile_pool(name="persist", bufs=1) as pp:
            nc.gpsimd.load_library(mlp)
            ident = pp.tile([P, P], f32)
            make_identity(nc, ident[:])
            bs_sb = pp.tile([128, T_pad], mybir.dt.uint8)
            dr_sb = pp.tile([128, T_pad], f32)
            wc_sb = pp.tile([64, 32], f32)
            w0_sb = pp.tile([64, 32], f32)
            b0_sb = pp.tile([P, 32], f32)
            cb_sb = pp.tile([P, 32], f32)
            w1_sb = pp.tile([128, 8], f32)
            b1_sb = pp.tile([P, T_B * 8], f32)
            w2_sb = pp.tile([P, T_B * 8], f32)
            iof_sb = pp.tile([P, P], f32)
            iop_sb = pp.tile([P, P], f32)
            on1_sb = pp.tile([1, P], f32)
            b2_sb = pp.tile([P, 1], f32)
            for sb, d in [(bs_sb, bs_d), (dr_sb, dr_d),
                          (wc_sb, wc_d), (w0_sb, w0_d), (b0_sb, b0_d),
                          (cb_sb, cb_d), (w1_sb, w1_d), (b1_sb, b1_d),
                          (w2_sb, w2_d), (iof_sb, io_f_d), (iop_sb, io_p_d),
                          (on1_sb, on1_d), (b2_sb, b2_d)]:
                nc.sync.dma_start(out=sb[:], in_=d[:])
            hxT = pp.tile([64, BLOCKS * 128], f32)     # [g0^T; h^T] per block
            out_tab = pp.tile([P, BLOCKS * 32], f32)   # out rows, own shard
            A_str = pp.tile([P, T_pad * 32], f32)      # selected src rows

            def pair_dma(dram_flat, b, sb_tile):
                n = 128 if b < BLOCKS - 1 else LAST_BLK_N
                nc.sync.dma_start(
                    out=dram_flat[b * 4096:b * 4096 + n * 32]
                    .rearrange("(p f) -> p f", f=32),
                    in_=sb_tile[:n, :])

            for _rep in range(k_rep):
                # ---------------- phase 0: h = relu(x @ w0 + b0) -----------
                with (
                    tc.tile_pool(name="p0", bufs=3) as p0,
                    tc.tile_pool(name="p0p", bufs=2, space="PSUM") as p0p,
                ):
                    for b in range(BLOCKS):
                        xt = p0.tile([P, IN_FEAT], f32, tag="xt")
                        nc.sync.dma_start(
                            out=xt[:], in_=x_d[b * 128:(b + 1) * 128, :])
                        ps_xT = p0p.tile([IN_FEAT, P], f32, tag="xT")
                        nc.tensor.transpose(out=ps_xT[:], in_=xt[:],
                                            identity=ident[:])
                        xT = p0.tile([IN_FEAT, P], f32, tag="xTs")
                        nc.vector.tensor_copy(out=xT[:], in_=ps_xT[:])
                        ps_h = p0p.tile([P, 32], f32, tag="h")
                        nc.tensor.matmul(out=ps_h[:], lhsT=xT[:],
                                         rhs=w0_sb[:], start=True, stop=True)
                        hb = p0.tile([P, 32], f32, tag="hb")
                        nc.vector.tensor_tensor(out=hb[:], in0=ps_h[:],
                                                in1=b0_sb[:], op=add)
                        hs = p0.tile([P, 32], f32, tag="hs")
                        nc.scalar.activation(out=hs[:], in_=hb[:], func=Relu)
                        ps_hT = p0p.tile([32, P], f32, tag="hT")
                        nc.tensor.transpose(out=ps_hT[:], in_=hs[:],
                                            identity=ident[:])
                        nc.vector.tensor_copy(
                            out=hxT[32:64, b * 128:(b + 1) * 128],
                            in_=ps_hT[:])
                        pair_dma(h_shard, b, hs)
                nc.gpsimd.collective_compute(
                    "AllGather", bypass, groups,
                    ins=[h_shard[:]], outs=[h_full[:]])

                if phases < 1:
                    continue
                # ------- phase 1: gather h[src], aggregate, local out ------
                with (
                    tc.tile_pool(name="p1", bufs=3) as p1,
                    tc.tile_pool(name="p1b", bufs=3) as p1b,
                    tc.tile_pool(name="p1p", bufs=2, space="PSUM") as p1p,
                ):
                    ps_g0 = None
                    for op in range(n_ops):
                        gi = p1.tile([128, OP_T * 8], mybir.dt.int16,
                                     tag="gi")
                        nc.sync.dma_start(
                            out=gi[:],
                            in_=gsrc_d[:, op * OP_T * 8:(op + 1) * OP_T * 8])
                        gd = p1.tile([P, OP_T, 64], f32, tag="gd")
                        nc.gpsimd.dma_gather(
                            gd[:], h_full[:], gi[:],
                            OP_T * 128, OP_T * 128, 64)
                        for i in range(OP_T):
                            t = op * OP_T + i
                            if t >= T_total:
                                break
                            asl = A_str[:, t * 32:(t + 1) * 32]
                            nc.vector.tensor_copy(out=asl,
                                                  in_=gd[:, i, 0:32])
                            mk = bs_sb[:, t:t + 1].to_broadcast([P, 32])
                            nc.vector.copy_predicated(out=asl, mask=mk,
                                                      data=gd[:, i, 32:64])
                            b, j = divmod(t, T_B)
                            if j == 0:
                                ps_g0 = p1p.tile([P, 32], f32, tag="g0")
                            oh = p1b.tile([P, P], f32, tag="oh")
                            nc.vector.tensor_tensor(
                                out=oh[:],
                                in0=dr_sb[:, t:t + 1].to_broadcast([P, P]),
                                in1=iof_sb[:], op=iseq)
                            nc.tensor.matmul(
                                out=ps_g0[:], lhsT=oh[:],
                                rhs=A_str[:, t * 32:(t + 1) * 32],
                                start=(j == 0), stop=(j == T_B - 1))
                            if j == T_B - 1:
                                g0s = p1b.tile([P, 32], f32, tag="g0s")
                                nc.vector.tensor_copy(out=g0s[:], in_=ps_g0[:])
                                ps_t = p1p.tile([32, P], f32, tag="g0T")
                                nc.tensor.transpose(out=ps_t[:], in_=g0s[:],
                                                    identity=ident[:])
                                nc.vector.tensor_copy(
                                    out=hxT[0:32, b * 128:(b + 1) * 128],
                                    in_=ps_t[:])
                                ps_o = p1p.tile([P, 32], f32, tag="op")
                                nc.tensor.matmul(
                                    out=ps_o[:],
                                    lhsT=hxT[:, b * 128:(b + 1) * 128],
                                    rhs=wc_sb[:], start=True, stop=True)
                                ot = out_tab[:, b * 32:(b + 1) * 32]
                                nc.vector.tensor_tensor(
                                    out=ot, in0=ps_o[:], in1=cb_sb[:], op=add)
                                osnap = p1b.tile([P, 32], f32, tag="osnap")
                                nc.vector.tensor_copy(out=osnap[:], in_=ot)
                                pair_dma(o_shard, b, osnap)
                nc.gpsimd.collective_compute(
                    "AllGather", bypass, groups,
                    ins=[o_shard[:]], outs=[o_full[:]])

                if phases < 2:
                    continue
                # ---------------- phase 2: edge scores ---------------------
                with (
                    tc.tile_pool(name="p2", bufs=3) as p2,
                    tc.tile_pool(name="p2b", bufs=3) as p2b,
                    tc.tile_pool(name="p2p", bufs=2, space="PSUM") as p2p,
                    tc.tile_pool(name="p2q", bufs=2, space="PSUM") as p2q,
                ):
                    def block_tail(b):
                        if p2step < 1:
                            return
                        drt = p2b.tile([1, T_B * 128], f32, tag="drt")
                        nc.sync.dma_start(out=drt[:], in_=drT_d[b:b + 1, :])
                        bc = p2b.tile([P, T_B * 128], f32, tag="bc")
                        for c0 in range(0, T_B * 128, 512):
                            cn = min(512, T_B * 128 - c0)
                            ps_bc = p2q.tile([P, 512], f32, tag="bc")
                            nc.tensor.matmul(out=ps_bc[:, :cn],
                                             lhsT=on1_sb[:],
                                             rhs=drt[0:1, c0:c0 + cn],
                                             start=True, stop=True)
                            nc.vector.tensor_copy(out=bc[:, c0:c0 + cn],
                                                  in_=ps_bc[:, :cn])
                        if p2step < 2:
                            return
                        ps_B = p2p.tile([P, T_B * 32], f32, tag="B")
                        for i in range(T_B):
                            ohT = p2b.tile([P, P], f32, tag="ohT")
                            nc.vector.tensor_tensor(
                                out=ohT[:], in0=iop_sb[:],
                                in1=bc[:, i * 128:(i + 1) * 128], op=iseq)
                            nc.tensor.matmul(
                                out=ps_B[:, i * 32:(i + 1) * 32],
                                lhsT=ohT[:],
                                rhs=out_tab[:, b * 32:(b + 1) * 32],
                                start=True, stop=True)
                        if p2step < 3:
                            return
                        z = p2.tile([P, T_B * 32], f32, tag="z")
                        nc.vector.tensor_tensor(
                            out=z[:],
                            in0=A_str[:, b * T_B * 32:(b + 1) * T_B * 32],
                            in1=ps_B[:], op=mult)
                        if p2step < 4:
                            return
                        ps_m = p2p.tile([P, T_B * 8], f32, tag="m")
                        mi = 0
                        for c0 in range(0, T_B * 32, 96):
                            cn = min(96, T_B * 32 - c0)
                            ps_zT = p2q.tile([96, P], f32, tag="zT")
                            nc.tensor.transpose(out=ps_zT[:cn, :],
                                                in_=z[:, c0:c0 + cn],
                                                identity=ident[:])
                            if p2step < 5:
                                continue
                            for j0 in range(0, cn, 32):
                                zT32 = p2.tile([32, P], f32, tag="zTs")
                                nc.vector.tensor_copy(
                                    out=zT32[:], in_=ps_zT[j0:j0 + 32, :])
                                nc.tensor.matmul(
                                    out=ps_m[:, mi * 8:(mi + 1) * 8],
                                    lhsT=zT32[:], rhs=w1_sb[0:32, :],
                                    start=True, stop=True)
                                mi += 1
                        if p2step < 6:
                            return
                        s1 = p2.tile([P, T_B * 8], f32, tag="s1")
                        nc.vector.tensor_tensor(out=s1[:], in0=ps_m[:],
                                                in1=b1_sb[:], op=add)
                        s1r = p2.tile([P, T_B * 8], f32, tag="s1r")
                        nc.scalar.activation(out=s1r[:], in_=s1[:], func=Relu)
                        nc.vector.tensor_tensor(out=s1r[:], in0=s1r[:],
                                                in1=w2_sb[:], op=mult)
                        sc = p2.tile([P, T_B], f32, tag="sc")
                        nc.vector.reduce_sum(
                            out=sc[:],
                            in_=s1r[:].rearrange("p (t e) -> p t e", e=8),
                            axis=X)
                        sc2 = p2.tile([P, T_B], f32, tag="sc2")
                        nc.vector.tensor_tensor(
                            out=sc2[:], in0=sc[:],
                            in1=b2_sb[:, 0:1].to_broadcast([P, T_B]),
                            op=add)
                        nc.sync.dma_start(
                            out=sc_d[:, b * T_B:(b + 1) * T_B], in_=sc2[:])

                    for op in range(n_ops):
                        gi = p2.tile([128, OP_T * 8], mybir.dt.int16,
                                     tag="gi2")
                        nc.sync.dma_start(
                            out=gi[:],
                            in_=gsrc_d[:, op * OP_T * 8:(op + 1) * OP_T * 8])
                        gd = p2.tile([P, OP_T, 64], f32, tag="gd2")
                        nc.gpsimd.dma_gather(
                            gd[:], o_full[:], gi[:],
                            OP_T * 128, OP_T * 128, 64)
                        for i in range(OP_T):
                            t = op * OP_T + i
                            if t >= T_total:
                                break
                            asl = A_str[:, t * 32:(t + 1) * 32]
                            nc.vector.tensor_copy(out=asl,
                                                  in_=gd[:, i, 0:32])
                            mk = bs_sb[:, t:t + 1].to_broadcast([P, 32])
                            nc.vector.copy_predicated(out=asl, mask=mk,
                                                      data=gd[:, i, 32:64])
                            b, j = divmod(t, T_B)
                            if j == T_B - 1:
                                block_tail(b)
    nc.compile()
    return nc


def _in_maps(prep, wts):
    maps = []
    for c in range(N_CORES):
        maps.append({
            "x_sh": prep["x_sh"][c],
            "gsrc": prep["gsrc"][c],
            "bs_pt": prep["bs_pt"][c],
            "dr_pt": prep["dr_pt"][c],
            "drT": prep["drT"][c],
            "Wcat": wts["Wcat"], "w0": wts["w0"], "b0": wts["b0"],
            "cb": wts["cb"], "w1": wts["w1"], "b1": wts["b1"],
            "w2": wts["w2"], "iota_f": wts["iota_f"],
            "iota_p": wts["iota_p"], "ones1": wts["ones1"],
            "b2": wts["b2"],
        })
    return maps


def _assemble(results, prep):
    scores = np.empty(N_EDGES, np.float32)
    for c in range(N_CORES):
        flat = results[c]["scores"].T.reshape(-1)  # slot-major: t*128+p
        inv = prep["inv"][c]
        m = inv >= 0
        scores[inv[m]] = flat[m]
    return scores


def kernel(**inputs):
    from concourse.bass_utils import run_bass_kernel_spmd
    prep = _prep(inputs["x"], inputs["edge_index"])
    wts = _weights(inputs, prep["T_B"])
    nc = _build(prep["T_B"], prep["n_ops"], k_rep=1)
    res = run_bass_kernel_spmd(nc, _in_maps(prep, wts),
                               list(range(N_CORES)))
    return _assemble(res.results, prep)

